# revision 11
# baseline (speedup 1.0000x reference)
"""Folded-causal sequence-sharded MHA (B=2, T=2048, C=1024, H=16, HS=64).

Core c handles batch b=c//4 and j=c%4. Per batch, the 2048 query rows form 16
chunks of 128; core j owns chunks {4m+3-j : m=0..3} ("slot" m). Chunk 4m+3-j
needs keys s < 128*(4m+4-j); the program uniformly computes N_m = 4(m+1)
s-blocks for slot m (exact for j=0, up to 3 slack blocks otherwise, masked via
data). Total S/PV/exp work is 0.625x of the full rectangle, identical on all
cores. Keys stay in natural order; queries are host-gathered into slot order
(input xq) and the output is host-scattered back.

S is computed transposed per s-block over all slots that need it: for s-block
sb, slots m >= sb//4 are live, so rhs = qt[:, (sb//4)*128 : 512] keeps matmul
free sizes at 512/384/256/128. The causal mask only ever applies to the FIRST
live slot of an s-block and is slot-independent:
mask[jj=sb%4, p, t] = (128*jj + p - t <= 128*(3-j)).
"""

import os

import numpy as np
import ml_dtypes

B, T, C, NH, HS = 2, 2048, 1024, 16, 64
TO = T // 4  # own query rows per core
P = 128
CCH = C // P
NCORES = 8
SCALE = 1.0 / float(np.sqrt(C))

LAST_EXEC_NS = None
LAST_RESULTS = None
LAST_IN_MAPS = None

_PROGRAM_CACHE = {}


def _build_program(nreps=1, parts='all', unroll=False):
    import contextlib
    import concourse.mybir as mybir
    import concourse.tile as tile
    from concourse import bacc

    DT = mybir.dt.bfloat16
    F32 = mybir.dt.float32

    nc = bacc.Bacc("TRN2", target_bir_lowering=False, debug=False,
                   num_devices=NCORES)

    xT = nc.dram_tensor("xT", [C, T], DT, kind="ExternalInput").ap()
    xq = nc.dram_tensor("xq", [C, TO], DT, kind="ExternalInput").ap()
    wq = nc.dram_tensor("wq", [C, C], DT, kind="ExternalInput").ap()
    wk = nc.dram_tensor("wk", [C, C], DT, kind="ExternalInput").ap()
    wv = nc.dram_tensor("wv", [C, C], DT, kind="ExternalInput").ap()
    wo = nc.dram_tensor("wo", [C, C], DT, kind="ExternalInput").ap()
    # causal mask, slot-independent: [jj, 2(head bcast), p, t] -> sbuf [p, jj, 2, t]
    dmask = nc.dram_tensor("dmask", [P, 4, 2, P], DT, kind="ExternalInput").ap()
    out = nc.dram_tensor("out", [TO, C], F32, kind="ExternalOutput").ap()

    NSB = T // P  # 16 s-blocks

    with tile.TileContext(nc) as tc:
        with (
            tc.tile_pool(name="const", bufs=1) as const,
            tc.tile_pool(name="wpool", bufs=1) as wpool,
            tc.tile_pool(name="ppool", bufs=4) as ppool,
            tc.tile_pool(name="opool", bufs=3) as opool,
            tc.tile_pool(name="small", bufs=4) as small,
            tc.tile_pool(name="ps_qkv", bufs=2, space="PSUM") as ps_qkv,
            tc.tile_pool(name="ps_s", bufs=2, space="PSUM") as ps_s,
            tc.tile_pool(name="ps_o", bufs=1, space="PSUM") as ps_o,
        ):
          reps = range(nreps) if unroll else range(1)
          loop_cm = (tc.For_i(0, nreps, 1) if (nreps > 1 and not unroll)
                     else contextlib.nullcontext())
          with loop_cm:
           for rep in reps:
            sfx = f"_{rep}"
            # ---- resident tiles (Q-path loads first: xq + wq) -------------
            xqs = []
            for cc in range(CCH):
                t_ = const.tile([P, TO], DT, tag=f"xq{cc}")
                nc.sync.dma_start(out=t_, in_=xq[cc * P:(cc + 1) * P, :])
                xqs.append(t_)

            def load_w(dram, wtag):
                tiles = []
                for cc in range(CCH):
                    t_ = wpool.tile([P, C], DT, tag=f"{wtag}{cc}")
                    nc.sync.dma_start(out=t_, in_=dram[cc * P:(cc + 1) * P, :])
                    tiles.append(t_)
                return tiles

            w_q = load_w(wq, "wq")
            xt = []
            for cc in range(CCH):
                t_ = const.tile([P, T], DT, tag=f"xt{cc}")
                nc.sync.dma_start(out=t_, in_=xT[cc * P:(cc + 1) * P, :])
                xt.append(t_)
            kt = [const.tile([P, T], DT, tag=f"kt{i}", name=f"kt{i}{sfx}") for i in range(CCH)]
            vt = [const.tile([P, NH, HS + 1], DT, tag=f"vt{i}", name=f"vt{i}{sfx}")
                  for i in range(NSB)]
            qt = [const.tile([P, TO], DT, tag=f"qt{i}", name=f"qt{i}{sfx}") for i in range(CCH)]
            at = const.tile([P, CCH, TO], DT, tag="at")
            # mask: [p, jj, 2, t]
            dm = const.tile([P, 4, 2, P], DT, tag="dm")
            nc.sync.dma_start(out=dm, in_=dmask)

            # ---- stage 1: Q^T (own 512 rows, slot order) ------------------
            if parts == 'attn':
                for t_ in kt + vt + qt:
                    nc.vector.memset(t_, 0.5)
            for dc in range(CCH if parts != 'attn' else 0):
                ps = ps_qkv.tile([P, TO], F32)
                for cc in range(CCH):
                    nc.tensor.matmul(
                        ps,
                        lhsT=w_q[cc][:, dc * P:(dc + 1) * P],
                        rhs=xqs[cc],
                        start=(cc == 0), stop=(cc == CCH - 1),
                    )
                nc.vector.tensor_copy(qt[dc], ps)

            # ---- stage 2: V natural (+ones col) ---------------------------
            w_v = load_w(wv, "wv")
            for tb in range(NSB if parts != 'attn' else 0):
                for half in range(2):
                    ps = ps_qkv.tile([P, TO], F32)
                    for cc in range(CCH):
                        nc.tensor.matmul(
                            ps,
                            lhsT=xt[cc][:, tb * P:(tb + 1) * P],
                            rhs=w_v[cc][:, half * TO:(half + 1) * TO],
                            start=(cc == 0), stop=(cc == CCH - 1),
                        )
                    nc.vector.tensor_copy(
                        vt[tb][:, 8 * half:8 * half + 8, 0:HS],
                        ps.rearrange("p (h d) -> p h d", d=HS),
                    )
                nc.vector.memset(vt[tb][:, :, HS:HS + 1], 1.0)

            w_k = load_w(wk, "wk")
            w_o = load_w(wo, "wo")  # loaded early; consumed only by stage 5

            # ---- stage 3+4 interleaved ------------------------------------
            kt_state = {}

            def emit_kt_step(hp1, i):
                # two of the 32 K^T matmuls for head-pair hp1 (i in 0..15)
                if parts == 'attn':
                    return
                for jj in (2 * i, 2 * i + 1):
                    tch, cc = divmod(jj, CCH)
                    if cc == 0:
                        kt_state[tch] = ps_qkv.tile(
                            [P, TO], F32, tag="ps", name=f"kps{hp1}_{tch}{sfx}")
                    nc.tensor.matmul(
                        kt_state[tch],
                        lhsT=w_k[cc][:, hp1 * P:(hp1 + 1) * P],
                        rhs=xt[cc][:, tch * TO:(tch + 1) * TO],
                        start=(cc == 0), stop=(cc == CCH - 1),
                    )
                    if cc == CCH - 1:
                        nc.vector.tensor_copy(
                            kt[hp1][:, tch * TO:(tch + 1) * TO],
                            kt_state.pop(tch))

            for i in range(NSB):
                emit_kt_step(0, i)  # prologue: pair 0's K^T
            if parts == 'proj':
                nc.vector.memset(at, 0.25)
                for hp1 in range(1, NH // 2):
                    for i in range(NSB):
                        emit_kt_step(hp1, i)
            # chain steps: singles for quartiles 0-1, sb-pairs for quartile 2,
            # one quad for quartile 3 -- all share the [P, 2, 512] psum shape
            STEPS = [(0,), (1,), (2,), (3,), (4,), (5,), (6,), (7,),
                     (8, 9), (10, 11), (12, 13, 14, 15)]

            def step_layout(step):
                f = TO - (step[0] // 4) * P
                return [(sb, k * f, f) for k, sb in enumerate(step)]

            for hp in range(NH // 2 if parts != 'proj' else 0):
                h0, h1 = 2 * hp, 2 * hp + 1
                # single PSUM tile for both heads x 4 slots of PV output
                ot = ps_o.tile([HS + 1, 2, 4, P], F32, tag="ot")
                sps = {}
                ktc = 0

                def emit_s_step(si):
                    sp = ps_s.tile([P, 2, TO], F32, tag="sp", name=f"sp{hp}_{si}{sfx}")
                    for sb, col, f in step_layout(STEPS[si]):
                        lo = (sb // 4) * P
                        for hh in range(2):
                            nc.tensor.matmul(
                                sp[:, hh, col:col + f],
                                lhsT=kt[hp][hh * HS:(hh + 1) * HS,
                                            sb * P:(sb + 1) * P],
                                rhs=qt[hp][hh * HS:(hh + 1) * HS, lo:TO],
                                start=True, stop=True,
                            )
                    sps[si] = sp

                emit_s_step(0)
                emit_s_step(1)
                for si, step in enumerate(STEPS):
                    lay = step_layout(step)
                    width = sum(f for _, _, f in lay)
                    sp = sps.pop(si)
                    pt = ppool.tile([P, 2, TO], DT, tag="pt", name=f"pt{hp}_{si}{sfx}")
                    nc.scalar.activation(
                        pt[:, :, 0:width], sp[:, :, 0:width],
                        mybir.ActivationFunctionType.Exp, scale=SCALE)
                    if si + 2 < len(STEPS):
                        emit_s_step(si + 2)
                    # mask the first live slot's columns of each s-block
                    if len(step) == 4:
                        nc.vector.tensor_mul(
                            pt[:, :, 0:TO].rearrange("p h (jj t) -> p h jj t", t=P),
                            pt[:, :, 0:TO].rearrange("p h (jj t) -> p h jj t", t=P),
                            dm.rearrange("p jj h t -> p h jj t"))
                    else:
                        for sb, col, f in lay:
                            nc.vector.tensor_mul(
                                pt[:, :, col:col + P], pt[:, :, col:col + P],
                                dm[:, sb % 4, :, :])
                    for sb, col, f in lay:
                        q4 = sb // 4
                        for hh in range(2):
                            nc.tensor.matmul(
                                ot[:, hh, q4:4, :],
                                lhsT=vt[sb][:, (h0, h1)[hh], :],
                                rhs=pt[:, hh, col:col + f],
                                start=(sb == 0), stop=(sb == NSB - 1),
                                skip_group_check=True,
                            )
                    # normalize a slot as soon as its accumulation completes
                    for sb, col, f in lay:
                        if sb % 4 == 3:
                            m = sb // 4
                            for hh in range(2):
                                rsum = small.tile([1, P], F32, tag="rsum")
                                nc.vector.reciprocal(rsum, ot[HS:HS + 1, hh, m, :])
                                bcast = small.tile([HS, P], F32, tag="bcast")
                                nc.gpsimd.partition_broadcast(bcast, rsum, channels=HS)
                                nc.vector.tensor_mul(
                                    at[hh * HS:hh * HS + HS, hp, m * P:(m + 1) * P],
                                    ot[0:HS, hh, m, :], bcast)
                    if hp + 1 < NH // 2:
                        target = min(((si + 1) * NSB + len(STEPS) - 1) // len(STEPS), NSB)
                        while ktc < target:
                            emit_kt_step(hp + 1, ktc)
                            ktc += 1

            # ---- stage 5: output projection (own rows, slot order) --------
            for tb in range(TO // P):
                for half in range(2):
                    ps = ps_qkv.tile([P, TO], F32)
                    for cc in range(CCH):
                        nc.tensor.matmul(
                            ps,
                            lhsT=at[:, cc, tb * P:(tb + 1) * P],
                            rhs=w_o[cc][:, half * TO:(half + 1) * TO],
                            start=(cc == 0), stop=(cc == CCH - 1),
                        )
                    ob = opool.tile([P, TO], F32, tag="ob")
                    nc.vector.tensor_copy(ob, ps)
                    nc.sync.dma_start(
                        out=out[tb * P:(tb + 1) * P, half * TO:(half + 1) * TO],
                        in_=ob,
                    )

    nc.compile()
    return nc


def _get_program(nreps=1):
    key = ("nc", nreps)
    if key not in _PROGRAM_CACHE:
        _PROGRAM_CACHE[key] = _build_program(nreps)
    return _PROGRAM_CACHE[key]


def _chunks_for(j):
    # slot m -> chunk id 4m+3-j
    return [4 * m + 3 - j for m in range(4)]


def kernel(x, Wq, Wk, Wv, Wo):
    global LAST_EXEC_NS, LAST_RESULTS, LAST_IN_MAPS
    from concourse.bass_utils import run_bass_kernel_spmd

    bf16 = ml_dtypes.bfloat16
    x = np.asarray(x, dtype=np.float32)
    Wq = np.asarray(Wq, dtype=np.float32)
    Wk = np.asarray(Wk, dtype=np.float32)
    Wv = np.asarray(Wv, dtype=np.float32)
    Wo = np.asarray(Wo, dtype=np.float32)

    wq = np.ascontiguousarray(Wq.transpose(1, 0, 2).reshape(C, C)).astype(bf16)
    wk = np.ascontiguousarray(Wk.transpose(1, 0, 2).reshape(C, C)).astype(bf16)
    wv = np.ascontiguousarray(Wv.transpose(1, 0, 2).reshape(C, C)).astype(bf16)
    wo = np.ascontiguousarray(Wo.T).astype(bf16)

    in_maps = []
    for c in range(NCORES):
        b, j = divmod(c, 4)
        xTb = np.ascontiguousarray(x[b].T).astype(bf16)  # [C, T] natural
        chunks = _chunks_for(j)
        cols = np.concatenate([np.arange(P * ch, P * (ch + 1)) for ch in chunks])
        xqb = np.ascontiguousarray(xTb[:, cols])
        # mask[p, jj, 2, t] = 1 if 128*jj + p - t <= 128*(3-j)
        p_ = np.arange(P)[:, None, None, None]
        jj_ = np.arange(4)[None, :, None, None]
        t_ = np.arange(P)[None, None, None, :]
        dmask = ((P * jj_ + p_ - t_) <= P * (3 - j)).astype(bf16)
        dmask = np.ascontiguousarray(np.broadcast_to(dmask, (P, 4, 2, P)))
        in_maps.append({
            "xT": xTb, "xq": xqb, "wq": wq, "wk": wk, "wv": wv, "wo": wo,
            "dmask": dmask,
        })

    LAST_IN_MAPS = in_maps
    nc = _get_program()
    trace = os.environ.get("KERNEL_TRACE", "0") == "1"
    res = run_bass_kernel_spmd(nc, in_maps, list(range(NCORES)), trace=trace)
    LAST_EXEC_NS = res.exec_time_ns
    LAST_RESULTS = res

    outp = np.empty((B, T, C), dtype=np.float32)
    for c in range(NCORES):
        b, j = divmod(c, 4)
        for m, ch in enumerate(_chunks_for(j)):
            outp[b, P * ch:P * (ch + 1)] = res.results[c]["out"][P * m:P * (m + 1)]
    return outp
